# revision 1
# baseline (speedup 1.0000x reference)
"""Trainium2 Bass kernel for causal multi-head attention.

Problem: B=4, S=2048, D=1024, H=16 (head_dim 64), fp32.
  qkv = x @ w_attn + b_attn ; causal SDPA ; out @ w_proj + b_proj

Sharding (8 cores): data-parallel over B (4) x tensor-parallel over head
halves (2). Core c handles batch b=c//2, heads [8*(c%2), 8*(c%2)+8).
Each core computes its qkv slice, its heads' attention, and a partial
output projection (its heads' rows of w_proj); the host sums the two
partials per batch. b_proj is added on even cores (odd cores get zeros).

Device dataflow (per core):
  P1: qT,kT = (w_q|k)^T @ x^T  -> [e, s] layout (e on partitions);
      v = x @ w_v (natural [s, e]); biases + 1/sqrt(hd) fused into the
      PSUM->SBUF evacuations. v is stored augmented as [v_h | ones].
  P2: per head-pair p, query block g (512 wide), key tile t (128):
      scoresT = kT.T @ qT (2 heads row-tiled), exp on ACT (no max
      subtraction: scores are O(+-6)), block-causal via narrowed moving
      ranges + a triangular multiply on diagonal tiles, then
      [outT | den] += [v | ones].T @ expT  (fp32r requires col_grp=0xf,
      so den rides along in the same M=128 matmul, replicated over
      partitions 64:128). Normalization by 1/den applies to outT only.
  P3: y_partial = aoT.T @ w_proj_slice (+ b_proj), DMA out.

All matmuls run as float32r (E8M11 weights, full PE rate at moving
dim >= 256; measured matmul norm-rel error ~1e-7 on rounded inputs).
"""

import math
import os

import numpy as np

import concourse.bass as bass
import concourse.mybir as mybir
import concourse.tile as tile
from concourse import bacc

last_exec_time_ns = None

B, S, D, H = 4, 2048, 1024, 16
HD = D // H          # 64
HPC = H // 2         # heads per core = 8
EC = HPC * HD        # per-core qkv slice width = 512
NP = 4               # head pairs per core
QB = 512             # query block width
KT = 128             # key tile
N_QB = S // QB       # 4
N_KT = S // KT       # 16
DT = D // 128        # 8 contraction tiles

F32 = mybir.dt.float32
F32R = mybir.dt.float32r

_nc_cache: dict = {}


def _round_fp32r(x: np.ndarray) -> np.ndarray:
    """Round-to-nearest-even fp32 -> fp32r (E8M11: low 12 mantissa bits 0)."""
    u = np.ascontiguousarray(x, dtype=np.float32).view(np.uint32)
    u2 = (u + 0x7FF + ((u >> 12) & 1)) & np.uint32(0xFFFFF000)
    return u2.view(np.float32)


def _build(causal: bool):
    nc = bacc.Bacc("TRN2", target_bir_lowering=False)
    xT = nc.dram_tensor("xT", [D, S], F32R, kind="ExternalInput")
    wqkv = nc.dram_tensor("wqkv", [D, 3 * EC], F32R, kind="ExternalInput")
    bqk = nc.dram_tensor("bqk", [128, 2 * EC // 128], F32, kind="ExternalInput")
    bv = nc.dram_tensor("bv", [1, EC], F32, kind="ExternalInput")
    wp = nc.dram_tensor("wp", [EC, D], F32R, kind="ExternalInput")
    bp = nc.dram_tensor("bp", [1, D], F32, kind="ExternalInput")
    tri = nc.dram_tensor("tri", [128, 128], F32R, kind="ExternalInput")
    ones = nc.dram_tensor("ones", [128, 64], F32R, kind="ExternalInput")
    y = nc.dram_tensor("y", [S, D], F32, kind="ExternalOutput")

    n_qk_et = 2 * EC // 128   # 8 e-tiles for q+k
    scale = 1.0 / math.sqrt(HD)

    with tile.TileContext(nc) as tc:
        with tc.tile_pool(name="consts", bufs=1) as consts:
            bqk_sb = consts.tile([128, n_qk_et], F32, tag="bqk_sb")
            tri_sb = consts.tile([128, 128], F32R, tag="tri_sb")


            with (
                tc.tile_pool(name="qkvp", bufs=1) as qkvp,
                tc.tile_pool(name="psS", bufs=3, space="PSUM") as psS,
                tc.tile_pool(name="psO", bufs=1, space="PSUM") as psO,
            ):
                qT = qkvp.tile([128, NP, S], F32R, tag="qT")
                kT = qkvp.tile([128, NP, S], F32R, tag="kT")
                # augmented v: per head h and key tile t, [v_h | ones] so a
                # single M=128 matmul yields both attn@v and the softmax
                # denominator (replicated over partitions 64:128)
                va = qkvp.tile([128, N_KT, HPC, 128], F32R, tag="va")
                # ---------------- P1: qkv projections ----------------
                with (
                    tc.tile_pool(name="p1sb", bufs=2) as p1sb,
                    tc.tile_pool(name="p1w", bufs=1) as p1w,
                ):
                    bv_sb = p1w.tile([128, EC], F32, tag="bv_sb")
                    w_sb = p1w.tile([128, DT, 3 * EC], F32R, tag="w_sb")
                    for dt in range(DT):
                        eng = nc.sync if dt in (2, 5, 7) else nc.scalar
                        eng.dma_start(
                            out=w_sb[:, dt, :],
                            in_=wqkv.ap()[dt * 128:(dt + 1) * 128, :],
                        )
                    nc.scalar.dma_start(
                        out=bv_sb, in_=bv.ap().to_broadcast([128, EC])
                    )
                    nc.scalar.dma_start(out=bqk_sb, in_=bqk.ap())
                    CB = 256  # P1 s-chunk width (SBUF-constrained)
                    for sc in range(S // CB):
                        s0 = sc * CB
                        xts = p1sb.tile([128, DT, CB], F32R, tag="xts")
                        for dt in range(DT):
                            nc.sync.dma_start(
                                out=xts[:, dt, :],
                                in_=xT.ap()[dt * 128:(dt + 1) * 128, s0:s0 + CB],
                            )
                        # q,k: out [e-tile, s-chunk] accumulated over d
                        last = sc == S // CB - 1
                        for et in range(n_qk_et):
                            if last and et >= n_qk_et - 2:
                                pqk = psO.tile([128, CB], F32,
                                               tag="OA" if et % 2 else "OB",
                                               name="pqk")
                            else:
                                pqk = psS.tile([128, CB], F32, tag="SAB", name="pqk")
                            for dt in range(DT):
                                nc.tensor.matmul(
                                    pqk,
                                    w_sb[:, dt, et * 128:(et + 1) * 128],
                                    xts[:, dt, :],
                                    start=(dt == 0),
                                    stop=(dt == DT - 1),
                                )
                            dst = qT if et < NP else kT
                            slab = et if et < NP else et - NP
                            nc.vector.tensor_scalar(
                                out=dst[:, slab, s0:s0 + CB],
                                in0=pqk,
                                scalar1=bqk_sb[:, et:et + 1],
                                scalar2=scale if et < NP else 1.0,
                                op0=mybir.AluOpType.add,
                                op1=mybir.AluOpType.mult,
                            )
                        # v: natural layout [s-tile, e] accumulated over d
                        for st in range(CB // 128):
                            if last:
                                pv = psO.tile([128, EC], F32,
                                              tag="OA" if st % 2 else "OB",
                                              name="pv")
                            else:
                                pv = psS.tile([128, EC], F32, tag="SAB", name="pv")
                            for dt in range(DT):
                                nc.tensor.matmul(
                                    pv,
                                    xts[:, dt, st * 128:(st + 1) * 128],
                                    w_sb[:, dt, 2 * EC:3 * EC],
                                    start=(dt == 0),
                                    stop=(dt == DT - 1),
                                )
                            nc.vector.tensor_tensor(
                                out=va[:, sc * (CB // 128) + st, :, 64:128],
                                in0=pv.rearrange("p (h e) -> p h e", e=64),
                                in1=bv_sb.rearrange("p (h e) -> p h e", e=64),
                                op=mybir.AluOpType.add,
                            )

                # ones half of augmented v + triangular mask, loaded late so
                # they don't delay the P1 weight/activation DMAs
                nc.scalar.dma_start(out=tri_sb, in_=tri.ap())
                ones_ap = ones.ap()
                ones_b = bass.AP(
                    tensor=ones_ap.tensor,
                    offset=ones_ap.offset,
                    ap=[ones_ap.ap[0], [0, HPC], ones_ap.ap[1]],
                )
                for t in range(N_KT):
                    nc.scalar.dma_start(out=va[:, t, :, 0:64], in_=ones_b)

                # aoT pool opens after P1 so its 32KB per partition is
                # not charged against P1's budget (pools allocate at open)
                with tc.tile_pool(name="aop", bufs=1) as aop:
                    aoT = aop.tile([128, NP, S], F32R, tag="aoT")
                    w3 = tc.tile_pool(name="p3w", bufs=1)
                    p3w = w3.__enter__()
                    # ---------------- P2: attention ----------------
                    # p3 constants prefetched here so the P2->P3 transition
                    # has no DMA wait
                    bp_sb = p3w.tile([128, D], F32, tag="bp_sb")
                    nc.scalar.dma_start(
                        out=bp_sb, in_=bp.ap().to_broadcast([128, D])
                    )
                    wp_sb = p3w.tile([128, EC // 128, D], F32R, tag="wp_sb")
                    for eo in range(EC // 128):
                        nc.scalar.dma_start(
                            out=wp_sb[:, eo, :],
                            in_=wp.ap()[eo * 128:(eo + 1) * 128, :],
                        )
                    with (
                        tc.tile_pool(name="p2e", bufs=3) as p2e,
                        tc.tile_pool(name="p2r", bufs=1) as p2r,
                        tc.tile_pool(name="p3y", bufs=2) as p3y,
                    ):
                        tri_b = bass.AP(
                            tensor=tri_sb.tensor,
                            offset=tri_sb.offset,
                            ap=[tri_sb.ap[0], [0, 2], tri_sb.ap[1]],
                        )
                        LOOK = 2  # score/exp tiles emitted ahead of attn@v

                        def emit_p3_group(st, dh):
                            py = psS.tile([128, QB], F32, tag="SAB")
                            for eo in range(EC // 128):
                                nc.tensor.matmul(
                                    py,
                                    aoT[:, eo, st * 128:(st + 1) * 128],
                                    wp_sb[:, eo, dh * QB:(dh + 1) * QB],
                                    start=(eo == 0),
                                    stop=(eo == EC // 128 - 1),
                                )
                            ysb = ysb_tiles[st % 4]
                            nc.vector.tensor_tensor(
                                out=ysb[:, dh * QB:(dh + 1) * QB],
                                in0=py,
                                in1=bp_sb[:, dh * QB:(dh + 1) * QB],
                                op=mybir.AluOpType.add,
                            )
                            if dh == D // QB - 1:
                                nc.sync.dma_start(
                                    out=y.ap()[st * 128:(st + 1) * 128, :],
                                    in_=ysb,
                                )

                        p3_queue = []  # (st, dh) groups awaiting emission
                        ysb_tiles = {}
                        for g in range(N_QB):
                            q0 = g * QB
                            n_t = 4 * (g + 1) if causal else N_KT
                            for p in range(NP):
                                OA = psO.tile([128, QB], F32, tag="OA")
                                OB = psO.tile([128, QB], F32, tag="OB")

                                def emit_score_exp(t):
                                    j = t - 4 * g if causal else -1
                                    qlo = 128 * j if j >= 0 else 0
                                    SAB = psS.tile([128, 2, QB], F32, tag="SAB")
                                    k0 = t * KT
                                    nc.tensor.matmul(
                                        SAB[:, 0, qlo:],
                                        kT[0:64, p, k0:k0 + KT],
                                        qT[0:64, p, q0 + qlo:q0 + QB],
                                        start=True, stop=True,
                                    )
                                    nc.tensor.matmul(
                                        SAB[:, 1, qlo:],
                                        kT[64:128, p, k0:k0 + KT],
                                        qT[64:128, p, q0 + qlo:q0 + QB],
                                        start=True, stop=True,
                                    )
                                    eAB = p2e.tile([128, 2, QB], F32R, tag="eAB")
                                    nc.scalar.activation(
                                        eAB[:, :, qlo:], SAB[:, :, qlo:],
                                        mybir.ActivationFunctionType.Exp,
                                    )
                                    if j >= 0:
                                        nc.vector.tensor_tensor(
                                            out=eAB[:, :, qlo:qlo + 128],
                                            in0=eAB[:, :, qlo:qlo + 128],
                                            in1=tri_b,
                                            op=mybir.AluOpType.mult,
                                        )
                                    return qlo, eAB

                                def emit_av(t, qlo, eAB):
                                    nc.tensor.matmul(
                                        OA[:, qlo:],
                                        va[:, t, 2 * p, :],
                                        eAB[:, 0, qlo:],
                                        start=(t == 0), stop=(t == n_t - 1),
                                    )
                                    nc.tensor.matmul(
                                        OB[:, qlo:],
                                        va[:, t, 2 * p + 1, :],
                                        eAB[:, 1, qlo:],
                                        start=(t == 0), stop=(t == n_t - 1),
                                    )

                                pending = []
                                for t in range(n_t):
                                    pending.append((t, *emit_score_exp(t)))
                                    if len(pending) > LOOK:
                                        emit_av(*pending.pop(0))
                                for item in pending:
                                    emit_av(*item)

                                # PE filler while DVE normalizes: 2 deferred
                                # output-projection groups from the previous
                                # query block
                                for _ in range(2):
                                    if p3_queue:
                                        emit_p3_group(*p3_queue.pop(0))

                                rcpA = p2r.tile([64, QB], F32, tag="rcpA")
                                rcpB = p2r.tile([64, QB], F32, tag="rcpB")
                                nc.vector.reciprocal_approx_fast(
                                    out=rcpA, in_=OA[0:64, :]
                                )
                                nc.vector.reciprocal_approx_fast(
                                    out=rcpB, in_=OB[0:64, :]
                                )
                                nc.vector.tensor_tensor(
                                    out=aoT[0:64, p, q0:q0 + QB],
                                    in0=OA[64:128, :],
                                    in1=rcpA,
                                    op=mybir.AluOpType.mult,
                                )
                                nc.vector.tensor_tensor(
                                    out=aoT[64:128, p, q0:q0 + QB],
                                    in0=OB[64:128, :],
                                    in1=rcpB,
                                    op=mybir.AluOpType.mult,
                                )

                            # queue this block's output projection; allocate
                            # its ysb tiles now
                            for st in range(4 * g, 4 * (g + 1)):
                                ysb_tiles[st % 4] = p3y.tile(
                                    [128, D], F32, tag="ysb", name="ysb"
                                )
                                for dh in range(D // QB):
                                    p3_queue.append((st, dh))
                        while p3_queue:
                            emit_p3_group(*p3_queue.pop(0))
                    w3.__exit__(None, None, None)

    nc.compile()
    return nc


def _get_nc(causal: bool):
    if causal not in _nc_cache:
        _nc_cache[causal] = _build(causal)
    return _nc_cache[causal]


def _numpy_fallback(x, mask, w_attn, b_attn, w_proj, b_proj):
    x64 = x.astype(np.float64)
    qkv = x64 @ w_attn.astype(np.float64) + b_attn.astype(np.float64)
    q, k, v = np.split(qkv, 3, axis=-1)
    sp = lambda t: t.reshape(B, S, H, HD).transpose(0, 2, 1, 3)
    q, k, v = sp(q), sp(k), sp(v)
    scores = np.einsum("bhqd,bhkd->bhqk", q, k) / math.sqrt(HD)
    m = np.broadcast_to(np.asarray(mask, bool), scores.shape)
    scores = np.where(m, scores, -np.inf)
    scores -= scores.max(axis=-1, keepdims=True)
    e = np.exp(scores)
    attn = e / e.sum(axis=-1, keepdims=True)
    out = np.einsum("bhqk,bhkd->bhqd", attn, v)
    out = out.transpose(0, 2, 1, 3).reshape(B, S, D)
    return (out @ w_proj.astype(np.float64) + b_proj.astype(np.float64)).astype(
        np.float32
    )


def kernel(x, mask, w_attn, b_attn, w_proj, b_proj) -> np.ndarray:
    from concourse.bass_utils import run_bass_kernel_spmd

    x = np.asarray(x, dtype=np.float32)
    w_attn = np.asarray(w_attn, dtype=np.float32)
    b_attn = np.asarray(b_attn, dtype=np.float32)
    w_proj = np.asarray(w_proj, dtype=np.float32)
    b_proj = np.asarray(b_proj, dtype=np.float32)

    m2 = np.asarray(mask, dtype=bool).reshape(S, S)
    if np.array_equal(m2, np.tril(np.ones((S, S), dtype=bool))):
        causal = True
    elif m2.all():
        causal = False
    else:
        return _numpy_fallback(x, mask, w_attn, b_attn, w_proj, b_proj)

    nc = _get_nc(causal)

    tri_np = np.triu(np.ones((128, 128), dtype=np.float32))
    ones_np = np.ones((128, 64), dtype=np.float32)

    in_maps = []
    for c in range(8):
        b, hg = divmod(c, 2)
        e0 = hg * EC
        q_sl = slice(e0, e0 + EC)
        k_sl = slice(D + e0, D + e0 + EC)
        v_sl = slice(2 * D + e0, 2 * D + e0 + EC)
        wq = w_attn[:, q_sl]
        wk = w_attn[:, k_sl]
        wv = w_attn[:, v_sl]
        # device evac computes (q_psum + bias) * scale for q tiles, so the
        # raw biases are passed
        bqk_np = np.concatenate([b_attn[q_sl], b_attn[k_sl]]).reshape(
            2 * EC // 128, 128).T
        in_maps.append({
            "xT": _round_fp32r(x[b].T),
            "wqkv": _round_fp32r(np.concatenate([wq, wk, wv], axis=1)),
            "bqk": np.ascontiguousarray(bqk_np, dtype=np.float32),
            "bv": b_attn[v_sl].reshape(1, EC).copy(),
            "wp": _round_fp32r(w_proj[q_sl, :]),
            "bp": (b_proj if hg == 0 else np.zeros_like(b_proj)).reshape(1, D).copy(),
            "tri": tri_np,
            "ones": ones_np,
        })

    trace = os.environ.get("KERNEL_TRACE") == "1"
    res = run_bass_kernel_spmd(nc, in_maps, core_ids=list(range(8)), trace=trace)
    global last_exec_time_ns
    if res.exec_time_ns is not None:
        last_exec_time_ns = res.exec_time_ns
    parts = [res.results[c]["y"] for c in range(8)]
    out = np.empty((B, S, D), dtype=np.float32)
    for b in range(B):
        out[b] = parts[2 * b] + parts[2 * b + 1]
    return out



# revision 3
# speedup vs baseline: 1.1943x; 1.1943x over previous
"""Trainium2 Bass kernel for causal multi-head attention.

Problem: B=4, S=2048, D=1024, H=16 (head_dim 64), fp32.
  qkv = x @ w_attn + b_attn ; causal SDPA ; out @ w_proj + b_proj

Sharding (8 cores): data-parallel over B (4) x tensor-parallel over head
halves (2). Core c handles batch b=c//2, heads [8*(c%2), 8*(c%2)+8).
Each core computes its qkv slice, its heads' attention, and a partial
output projection (its heads' rows of w_proj); the host sums the two
partials per batch. b_proj is added on even cores (odd cores get zeros).

All matmul inputs are bf16 (rounded on host / at PSUM evacuation);
accumulation stays fp32 in PSUM, so the only error sources are input
rounding (~0.1% rms/element), well inside the 2e-2 rel-err budget.

Device dataflow (per core), emitted as one fused pipeline:
  P1 (chunked over s in 4 chunks of 512):
      qT,kT = (w_q|k)^T @ x^T  -> [e, s] (e on partitions), bias+scale
      fused into the PSUM evacuation; v natural [s, e], stored augmented
      as [ones | v_h] per head so a single M=128 attn@v matmul also
      yields the softmax denominator.  Weights live in SBUF in an
      et-major layout so the first matmul only needs a 256KB DMA.
  P2 (query blocks g of 512, interleaved with P1 chunks g+1.. and P3):
      per head-pair p, key tile t (128): scoresT = kT.T @ qT, exp on ACT
      (no max subtraction: scores are O(+-6)), block-causal via narrowed
      moving ranges + triangular multiply on diagonal tiles, then
      [den | outT] += [ones | v].T @ expT.  The exp on the ACT engine is
      the P2 pacer (~0.85ns/elem), so P1/P3 matmul groups are
      interleaved between attention tiles as PE filler.
  P3: y_partial = aoT.T @ w_proj_slice (+ b_proj), queued per query
      block and drained as filler during the next block.
"""

import math
import os
from collections import deque

import ml_dtypes
import numpy as np

import concourse.bass as bass
import concourse.mybir as mybir
import concourse.tile as tile
from concourse import bacc

last_exec_time_ns = None

B, S, D, H = 4, 2048, 1024, 16
HD = D // H          # 64
HPC = H // 2         # heads per core = 8
EC = HPC * HD        # per-core qkv slice width = 512
NP = 4               # head pairs per core
QB = 512             # query block width
KT = 128             # key tile
CB = 512             # P1 s-chunk width
NC_ = S // CB        # 4 chunks
N_KT = S // KT       # 16
DT = D // 128        # 8 contraction tiles

F32 = mybir.dt.float32
BF16 = mybir.dt.bfloat16
NPBF = ml_dtypes.bfloat16

_nc_cache: dict = {}


def _build(causal: bool):
    nc = bacc.Bacc("TRN2", target_bir_lowering=False)
    xp = nc.dram_tensor("xp", [NC_, 128, DT, CB], BF16, kind="ExternalInput")
    wqk = nc.dram_tensor("wqk", [8, 128, DT, 128], BF16, kind="ExternalInput")
    wv = nc.dram_tensor("wv", [DT, 128, EC], BF16, kind="ExternalInput")
    wpj = nc.dram_tensor("wpj", [4, 128, D], BF16, kind="ExternalInput")
    bqk = nc.dram_tensor("bqk", [128, 8], F32, kind="ExternalInput")
    bv = nc.dram_tensor("bv", [1, EC], F32, kind="ExternalInput")
    bp = nc.dram_tensor("bp", [1, D], F32, kind="ExternalInput")
    tri = nc.dram_tensor("tri", [128, 128], BF16, kind="ExternalInput")
    y = nc.dram_tensor("y", [S, D], F32, kind="ExternalOutput")

    scale = 1.0 / math.sqrt(HD)
    LOOK = 2  # score/exp tiles emitted ahead of attn@v

    with tile.TileContext(nc) as tc, nc.allow_low_precision(
        reason="bf16 attention pipeline; fp32 PSUM accumulation throughout"
    ):
        with (
            tc.tile_pool(name="sb", bufs=1) as sb,
            tc.tile_pool(name="eABp", bufs=3) as eABp,
            tc.tile_pool(name="ysbp", bufs=3) as ysbp,
            tc.tile_pool(name="rcpp", bufs=2) as rcpp,
            tc.tile_pool(name="p1ps", bufs=2, space="PSUM") as p1ps,
            tc.tile_pool(name="psS", bufs=2, space="PSUM") as psS,
            tc.tile_pool(name="psO", bufs=1, space="PSUM") as psO,
        ):
            w_qk = sb.tile([128, 8, DT, 128], BF16, tag="w_qk")
            w_v = sb.tile([128, DT, EC], BF16, tag="w_v")
            wp_sb = sb.tile([128, 4, D], BF16, tag="wp_sb")
            xts = sb.tile([128, NC_, DT, CB], BF16, tag="xts")
            qT = sb.tile([128, NP, S], BF16, tag="qT")
            kT = sb.tile([128, NP, S], BF16, tag="kT")
            # augmented v: per head h and key tile t, [ones | v_h] so one
            # M=128 matmul yields both attn@v and the softmax denominator
            va = sb.tile([128, N_KT, HPC, 128], BF16, tag="va")
            aoT = sb.tile([128, NP, S], BF16, tag="aoT")
            bqk_sb = sb.tile([128, 8], F32, tag="bqk_sb")
            bv_sb = sb.tile([128, EC], F32, tag="bv_sb")
            bp_sb = sb.tile([128, D], F32, tag="bp_sb")
            tri_sb = sb.tile([128, 128], BF16, tag="tri_sb")

            # ---- prologue DMAs, spread over non-critical queues ----
            # scalar queue is free until the first exp (~20us in)
            nc.scalar.dma_start(out=bqk_sb, in_=bqk.ap())
            nc.scalar.dma_start(out=bv_sb, in_=bv.ap().to_broadcast([128, EC]))
            nc.scalar.dma_start(out=tri_sb, in_=tri.ap())
            nc.scalar.dma_start(out=bp_sb, in_=bp.ap().to_broadcast([128, D]))
            # sync: weights, first-needed first (et0 slab is 256KB)
            for et in range(8):
                nc.sync.dma_start(out=w_qk[:, et], in_=wqk.ap()[et])
            for dt in range(DT):
                nc.sync.dma_start(out=w_v[:, dt], in_=wv.ap()[dt])
            # gpsimd: x chunks + ones memsets + P3 weights
            for c in range(NC_):
                nc.gpsimd.dma_start(out=xts[:, c], in_=xp.ap()[c])
                nc.gpsimd.memset(va[:, 4 * c:4 * c + 4, :, 0:64], 1.0)
            for eo in range(4):
                nc.gpsimd.dma_start(out=wp_sb[:, eo], in_=wpj.ap()[eo])

            # ---------------- P1: one chunk = 12 matmul groups ----------------
            def p1_groups(c):
                gs = []
                for et in range(8):
                    def qk_group(et=et, c=c):
                        ps = p1ps.tile([128, CB], F32, tag="P1", name=f"qk{c}_{et}")
                        for dt in range(DT):
                            nc.tensor.matmul(
                                ps,
                                w_qk[:, et, dt, :],
                                xts[:, c, dt, :],
                                start=(dt == 0),
                                stop=(dt == DT - 1),
                            )
                        dst, slab = (qT, et) if et < 4 else (kT, et - 4)
                        nc.vector.tensor_scalar(
                            out=dst[:, slab, c * CB:(c + 1) * CB],
                            in0=ps,
                            scalar1=bqk_sb[:, et:et + 1],
                            scalar2=scale if et < 4 else 1.0,
                            op0=mybir.AluOpType.add,
                            op1=mybir.AluOpType.mult,
                        )
                    gs.append(qk_group)
                for st in range(CB // 128):
                    def v_group(st=st, c=c):
                        ps = p1ps.tile([128, EC], F32, tag="P1", name=f"v{c}_{st}")
                        for dt in range(DT):
                            nc.tensor.matmul(
                                ps,
                                xts[:, c, dt, st * 128:(st + 1) * 128],
                                w_v[:, dt, :],
                                start=(dt == 0),
                                stop=(dt == DT - 1),
                            )
                        nc.vector.tensor_tensor(
                            out=va[:, c * 4 + st, :, 64:128],
                            in0=ps.rearrange("p (h e) -> p h e", e=64),
                            in1=bv_sb.rearrange("p (h e) -> p h e", e=64),
                            op=mybir.AluOpType.add,
                        )
                    gs.append(v_group)
                return gs

            # ---------------- P3: one group = 4 matmuls + bias/DMA ----------------
            ysb_tiles = {}

            def emit_p3(st, dh):
                py = p1ps.tile([128, QB], F32, tag="P1", name=f"p3_{st}_{dh}")
                for eo in range(4):
                    nc.tensor.matmul(
                        py,
                        aoT[:, eo, st * 128:(st + 1) * 128],
                        wp_sb[:, eo, dh * QB:(dh + 1) * QB],
                        start=(eo == 0),
                        stop=(eo == 3),
                    )
                ysb = ysb_tiles[st]
                nc.vector.tensor_tensor(
                    out=ysb[:, dh * QB:(dh + 1) * QB],
                    in0=py,
                    in1=bp_sb[:, dh * QB:(dh + 1) * QB],
                    op=mybir.AluOpType.add,
                )
                if dh == D // QB - 1:
                    nc.sync.dma_start(
                        out=y.ap()[st * 128:(st + 1) * 128, :], in_=ysb
                    )

            fillers = deque()   # pending P1 groups (callables)
            p3q = deque()       # pending P3 (st, dh) groups

            def pop_filler():
                if fillers:
                    fillers.popleft()()
                    return True
                if p3q:
                    emit_p3(*p3q.popleft())
                    return True
                return False

            tri_b = None

            def get_tri_b():
                nonlocal tri_b
                if tri_b is None:
                    tri_b = bass.AP(
                        tensor=tri_sb.tensor,
                        offset=tri_sb.offset,
                        ap=[tri_sb.ap[0], [0, 2], tri_sb.ap[1]],
                    )
                return tri_b

            # ---------------- P2: attention pair with PE filler ----------------
            def emit_pair(g, p, quota):
                q0 = g * QB
                n_t = 4 * (g + 1) if causal else N_KT
                OA = psO.tile([128, QB], F32, tag="OA")
                OB = psO.tile([128, QB], F32, tag="OB")

                def emit_score_exp(t):
                    j = t - 4 * g if causal else -1
                    qlo = 128 * j if j >= 0 else 0
                    SAB = psS.tile([128, 2, QB], F32, tag="SAB")
                    k0 = t * KT
                    nc.tensor.matmul(
                        SAB[:, 0, qlo:],
                        kT[0:64, p, k0:k0 + KT],
                        qT[0:64, p, q0 + qlo:q0 + QB],
                        start=True, stop=True,
                    )
                    nc.tensor.matmul(
                        SAB[:, 1, qlo:],
                        kT[64:128, p, k0:k0 + KT],
                        qT[64:128, p, q0 + qlo:q0 + QB],
                        start=True, stop=True,
                    )
                    eAB = eABp.tile([128, 2, QB], BF16, tag="eAB")
                    nc.scalar.activation(
                        eAB[:, :, qlo:], SAB[:, :, qlo:],
                        mybir.ActivationFunctionType.Exp,
                    )
                    if j >= 0:
                        nc.vector.tensor_tensor(
                            out=eAB[:, :, qlo:qlo + 128],
                            in0=eAB[:, :, qlo:qlo + 128],
                            in1=get_tri_b(),
                            op=mybir.AluOpType.mult,
                        )
                    return qlo, eAB

                def emit_av(t, qlo, eAB):
                    nc.tensor.matmul(
                        OA[:, qlo:],
                        va[:, t, 2 * p, :],
                        eAB[:, 0, qlo:],
                        start=(t == 0), stop=(t == n_t - 1),
                    )
                    nc.tensor.matmul(
                        OB[:, qlo:],
                        va[:, t, 2 * p + 1, :],
                        eAB[:, 1, qlo:],
                        start=(t == 0), stop=(t == n_t - 1),
                    )

                done = 0
                stride = max(1, n_t // quota) if quota else n_t + 1
                pending = []
                for t in range(n_t):
                    pending.append((t, *emit_score_exp(t)))
                    if done < quota and t % stride == stride - 1:
                        if pop_filler():
                            done += 1
                    if len(pending) > LOOK:
                        emit_av(*pending.pop(0))
                for item in pending:
                    emit_av(*item)
                while done < quota and pop_filler():
                    done += 1

                rcpA = rcpp.tile([64, QB], F32, tag="rcpA")
                rcpB = rcpp.tile([64, QB], F32, tag="rcpB")
                nc.vector.reciprocal_approx_fast(out=rcpA, in_=OA[0:64, :])
                nc.vector.reciprocal_approx_fast(out=rcpB, in_=OB[0:64, :])
                nc.vector.tensor_tensor(
                    out=aoT[0:64, p, q0:q0 + QB],
                    in0=OA[64:128, :],
                    in1=rcpA,
                    op=mybir.AluOpType.mult,
                )
                nc.vector.tensor_tensor(
                    out=aoT[64:128, p, q0:q0 + QB],
                    in0=OB[64:128, :],
                    in1=rcpB,
                    op=mybir.AluOpType.mult,
                )

            # ---------------- main schedule ----------------
            for fn in p1_groups(0):
                fn()
            if not causal:
                # every block needs all keys: run P1 fully first
                for c in range(1, NC_):
                    for fn in p1_groups(c):
                        fn()
            for g in range(NC_):
                if causal:
                    if g == 0:
                        fillers.extend(p1_groups(1))
                        fillers.extend(p1_groups(2))
                    elif g == 1:
                        fillers.extend(p1_groups(3))
                for p in range(NP):
                    remaining = len(fillers) + len(p3q)
                    quota = -(-remaining // (NP - p))  # ceil
                    emit_pair(g, p, quota)
                for st in range(4 * g, 4 * (g + 1)):
                    ysb_tiles[st] = ysbp.tile(
                        [128, D], F32, tag="ysb", name=f"ysb{st}"
                    )
                    for dh in range(D // QB):
                        p3q.append((st, dh))
            while p3q:
                emit_p3(*p3q.popleft())

    nc.compile()
    return nc


def _get_nc(causal: bool):
    if causal not in _nc_cache:
        _nc_cache[causal] = _build(causal)
    return _nc_cache[causal]


def _numpy_fallback(x, mask, w_attn, b_attn, w_proj, b_proj):
    x64 = x.astype(np.float64)
    qkv = x64 @ w_attn.astype(np.float64) + b_attn.astype(np.float64)
    q, k, v = np.split(qkv, 3, axis=-1)
    sp = lambda t: t.reshape(B, S, H, HD).transpose(0, 2, 1, 3)
    q, k, v = sp(q), sp(k), sp(v)
    scores = np.einsum("bhqd,bhkd->bhqk", q, k) / math.sqrt(HD)
    m = np.broadcast_to(np.asarray(mask, bool), scores.shape)
    scores = np.where(m, scores, -np.inf)
    scores -= scores.max(axis=-1, keepdims=True)
    e = np.exp(scores)
    attn = e / e.sum(axis=-1, keepdims=True)
    out = np.einsum("bhqk,bhkd->bhqd", attn, v)
    out = out.transpose(0, 2, 1, 3).reshape(B, S, D)
    return (out @ w_proj.astype(np.float64) + b_proj.astype(np.float64)).astype(
        np.float32
    )


def kernel(x, mask, w_attn, b_attn, w_proj, b_proj) -> np.ndarray:
    from concourse.bass_utils import run_bass_kernel_spmd

    x = np.asarray(x, dtype=np.float32)
    w_attn = np.asarray(w_attn, dtype=np.float32)
    b_attn = np.asarray(b_attn, dtype=np.float32)
    w_proj = np.asarray(w_proj, dtype=np.float32)
    b_proj = np.asarray(b_proj, dtype=np.float32)

    m2 = np.asarray(mask, dtype=bool).reshape(S, S)
    if np.array_equal(m2, np.tril(np.ones((S, S), dtype=bool))):
        causal = True
    elif m2.all():
        causal = False
    else:
        return _numpy_fallback(x, mask, w_attn, b_attn, w_proj, b_proj)

    nc = _get_nc(causal)

    tri_np = np.triu(np.ones((128, 128), dtype=np.float32)).astype(NPBF)

    in_maps = []
    for c in range(8):
        b, hg = divmod(c, 2)
        e0 = hg * EC
        q_sl = slice(e0, e0 + EC)
        k_sl = slice(D + e0, D + e0 + EC)
        v_sl = slice(2 * D + e0, 2 * D + e0 + EC)
        wqk_cat = np.concatenate([w_attn[:, q_sl], w_attn[:, k_sl]], axis=1)
        # [et][p][dt][e]: stationary tile (et, dt) rows = d in dt, cols = e
        wqk_p = np.ascontiguousarray(
            wqk_cat.reshape(DT, 128, 8, 128).transpose(2, 1, 0, 3)
        ).astype(NPBF)
        wv_p = np.ascontiguousarray(
            w_attn[:, v_sl].reshape(DT, 128, EC)
        ).astype(NPBF)
        wp_p = np.ascontiguousarray(
            w_proj[q_sl, :].reshape(4, 128, D)
        ).astype(NPBF)
        # [chunk][p][dt][s]
        xp_p = np.ascontiguousarray(
            x[b].T.reshape(DT, 128, NC_, CB).transpose(2, 1, 0, 3)
        ).astype(NPBF)
        # device evac computes (q_psum + bias) * scale for q tiles
        bqk_np = np.concatenate([b_attn[q_sl], b_attn[k_sl]]).reshape(8, 128).T
        in_maps.append({
            "xp": xp_p,
            "wqk": wqk_p,
            "wv": wv_p,
            "wpj": wp_p,
            "bqk": np.ascontiguousarray(bqk_np, dtype=np.float32),
            "bv": b_attn[v_sl].reshape(1, EC).astype(np.float32),
            "bp": (b_proj if hg == 0 else np.zeros_like(b_proj)).reshape(1, D)
                  .astype(np.float32),
            "tri": tri_np,
        })

    trace = os.environ.get("KERNEL_TRACE") == "1"
    res = run_bass_kernel_spmd(nc, in_maps, core_ids=list(range(8)), trace=trace)
    global last_exec_time_ns
    if res.exec_time_ns is not None:
        last_exec_time_ns = res.exec_time_ns
    parts = [res.results[c]["y"] for c in range(8)]
    out = np.empty((B, S, D), dtype=np.float32)
    for b in range(B):
        out[b] = parts[2 * b] + parts[2 * b + 1]
    return out
